# revision 1
# baseline (speedup 1.0000x reference)
"""Trainium2 Bass kernel for nn_DecomposingAttnProcessor.

Math (reference):
    q = hs @ Wq.T + bq;  k = ehs @ Wk.T + bk;  v = ehs @ Wv.T + bv
    scores = (q @ k.T) * dh**-0.5 per (bc, head)      [BC, H, S, T]
    w = softmax(scores over the COMPONENT axis)        (bc = c*B + b, C=4, B=2)
    w = w / (sum_t w + eps)
    out = (w @ v) -> [BC, S, D] -> @ Wo.T + bo

Distribution: shard S (4096 query tokens) across 8 cores, 512 each. Every
core handles all (bc, head) pairs for its S-slice, so the component softmax
group (same b, all c) stays on one core and each core emits complete output
rows (no cross-core reduction).

Layout strategy: everything stays transposed (host pre-transposes inputs):
    hsT [din, S], W*T [din, dout]  ->  qT [dout, S] (scaled by dh**-0.5)
    kT [dout, T], v [T, dv]        ->  scoresT [T, S] = kT_h.T-slice matmuls
    softmax elementwise on [T, S] tiles; row-normalization via ones-matmul
    rowsums [64, S] + reciprocal + multiply of the AV output outT [dh, S].
    attnT [f, S] feeds the O-projection directly as the moving operand:
    outT [dout, S]. The host un-transposes the final output.

All matmuls run in float32r (TF32, 1 cycle/row at N>=256 incl. K=64).
The w/v path of the AV + rowsum matmuls is bf16 (exp writes bf16 directly).
"""

import numpy as np
from contextlib import ExitStack

import concourse.bass as bass
import concourse.tile as tile
from concourse import bacc, mybir

F32 = mybir.dt.float32
F32R = mybir.dt.float32r
BF16 = mybir.dt.bfloat16

# problem shape (hardcoded per contract)
BC, S, D = 8, 4096, 1536
T = 154
C, B = 4, 2
H, DH = 24, 64
NCORES = 8
SL = S // NCORES          # 512 S-rows per core
NDI = D // 128            # 12 din chunks
NDO = D // 128            # 12 dout tiles
TP = 160                  # padded T stride in ehsT packing
T0, T1 = 128, T - 128     # T chunks: 128 + 26
SCALE = DH ** -0.5
import os
USE_GP_ADD = os.environ.get("DK_GP_ADD", "1") == "1"
USE_GP_MUL = os.environ.get("DK_GP_MUL", "1") == "1"
KV_N = BC * TP            # 1280
KV_BLOCKS = [(0, 512), (512, 512), (1024, 256)]   # N-blocks for kT proj
DV_BLOCKS = [(0, 512), (512, 512), (1024, 512)]   # N-blocks over D


def build_program():
    nc = bacc.Bacc("TRN2", target_bir_lowering=False, debug=False)

    # ---- external I/O (per core) ----
    hsT = nc.dram_tensor("hsT", [BC, D, SL], F32R, kind="ExternalInput").ap()
    ehsT = nc.dram_tensor("ehsT", [D, KV_N], F32R, kind="ExternalInput").ap()
    wqT = nc.dram_tensor("wqT", [D, D], F32R, kind="ExternalInput").ap()
    wkT = nc.dram_tensor("wkT", [D, D], F32R, kind="ExternalInput").ap()
    wvT = nc.dram_tensor("wvT", [D, D], F32R, kind="ExternalInput").ap()
    woT = nc.dram_tensor("woT", [D, D], F32R, kind="ExternalInput").ap()
    # biases laid out [128, 12] on host (bq pre-scaled by dh**-0.5)
    bqs = nc.dram_tensor("bqs", [128, NDO], F32, kind="ExternalInput").ap()
    bks = nc.dram_tensor("bks", [128, NDO], F32, kind="ExternalInput").ap()
    bos = nc.dram_tensor("bos", [128, NDO], F32, kind="ExternalInput").ap()
    bvr = nc.dram_tensor("bvr", [1, D], F32R, kind="ExternalInput").ap()
    onesr = nc.dram_tensor("onesr", [1, 128], F32R, kind="ExternalInput").ap()
    outT = nc.dram_tensor("outT", [BC, D, SL], F32, kind="ExternalOutput").ap()

    # ---- DRAM scratch (split per bc / per tile for fine-grained deps) ----
    kT_s = [nc.dram_tensor(f"kT_s{j}", [128, KV_N], F32R).ap()
            for j in range(NDO)]
    v_s = [nc.dram_tensor(f"v_s{bc}", [T, D], BF16).ap() for bc in range(BC)]
    qT_s = [nc.dram_tensor(f"qT_s{bc}", [NDO, 128, SL], F32R).ap()
            for bc in range(BC)]
    at_s = [nc.dram_tensor(f"at_s{bc}", [D, SL], F32R).ap()
            for bc in range(BC)]

    ENG_ADD = nc.gpsimd if USE_GP_ADD else nc.vector
    ENG_MUL = nc.gpsimd if USE_GP_MUL else nc.vector

    with tile.TileContext(nc) as tc, ExitStack() as ctx:
        const = ctx.enter_context(tc.tile_pool(name="const", bufs=1))
        ones_bf = const.tile([128, DH], BF16)
        nc.vector.memset(ones_bf[:], 1.0)
        ones_row = const.tile([1, 128], F32R)
        nc.sync.dma_start(ones_row[:], onesr[:])
        bq_t = const.tile([128, NDO], F32)
        bk_t = const.tile([128, NDO], F32)
        bo_t = const.tile([128, NDO], F32)
        bv_t = const.tile([1, D], F32R)
        nc.sync.dma_start(bq_t[:], bqs[:])
        nc.sync.dma_start(bk_t[:], bks[:])
        nc.sync.dma_start(bo_t[:], bos[:])
        nc.sync.dma_start(bv_t[:], bvr[:])

        # ================= P1: K and V projections =================
        with tc.tile_pool(name="eh", bufs=1) as ehp:
            eh = [ehp.tile([128, KV_N], F32R, tag=f"eh{_i}", name=f"eh{_i}") for _i in range(NDI)]
            for i in range(NDI):
                nc.sync.dma_start(eh[i][:], ehsT[i * 128:(i + 1) * 128, :])

            # ---- P1a: kT = Wk @ ehs^T (+bk), all bc packed along free ----
            with tc.tile_pool(name="p1a_w", bufs=1) as wp, \
                 tc.tile_pool(name="p1a_ps", bufs=4, space="PSUM") as pp, \
                 tc.tile_pool(name="p1a_o", bufs=3) as op:
                wk = [wp.tile([128, D], F32R, tag=f"wk{_i}", name=f"wk{_i}") for _i in range(NDI)]
                for i in range(NDI):
                    nc.sync.dma_start(wk[i][:], wkT[i * 128:(i + 1) * 128, :])
                for j in range(NDO):
                    for (nb0, nbl) in KV_BLOCKS:
                        ps = pp.tile([128, 512], F32, tag="ps")
                        for i in range(NDI):
                            nc.tensor.matmul(
                                ps[:, 0:nbl],
                                wk[i][:, j * 128:(j + 1) * 128],
                                eh[i][:, nb0:nb0 + nbl],
                                start=(i == 0), stop=(i == NDI - 1))
                        ot = op.tile([128, 512], F32R, tag="ot")
                        nc.scalar.activation(
                            ot[:, 0:nbl], ps[:, 0:nbl],
                            mybir.ActivationFunctionType.Identity,
                            bias=bk_t[:, j:j + 1])
                        nc.sync.dma_start(kT_s[j][:, nb0:nb0 + nbl],
                                          ot[:, 0:nbl])

            # ---- P1b: v = ehs @ Wv.T (+bv), stored bf16 ----
            with tc.tile_pool(name="p1b_w", bufs=1) as wp, \
                 tc.tile_pool(name="p1b_ps", bufs=4, space="PSUM") as pp, \
                 tc.tile_pool(name="p1b_o", bufs=3) as op:
                wv = [wp.tile([128, D], F32R, tag=f"wv{_i}", name=f"wv{_i}") for _i in range(NDI)]
                for i in range(NDI):
                    nc.sync.dma_start(wv[i][:], wvT[i * 128:(i + 1) * 128, :])
                for bc in (0, 2, 4, 6, 1, 3, 5, 7):
                    for (tt0, ttl) in [(0, T0), (T0, T1)]:
                        for (nb0, nbl) in DV_BLOCKS:
                            ps = pp.tile([128, 512], F32, tag="ps")
                            for i in range(NDI):
                                nc.tensor.matmul(
                                    ps[0:ttl, :],
                                    eh[i][:, bc * TP + tt0:bc * TP + tt0 + ttl],
                                    wv[i][:, nb0:nb0 + nbl],
                                    start=(i == 0), stop=False)
                            # bias row: K=1 matmul with ones-row lhsT
                            nc.tensor.matmul(
                                ps[0:ttl, :],
                                ones_row[0:1, 0:ttl],
                                bv_t[0:1, nb0:nb0 + nbl],
                                start=False, stop=True,
                                skip_group_check=True)
                            ot = op.tile([128, 512], BF16, tag="ot")
                            nc.scalar.copy(ot[0:ttl, :], ps[0:ttl, :])
                            nc.sync.dma_start(
                                v_s[bc][tt0:tt0 + ttl, nb0:nb0 + nbl],
                                ot[0:ttl, :])

        # ================= P2: Q projection (scaled) =================
        with tc.tile_pool(name="p2_w", bufs=1) as wp, \
             tc.tile_pool(name="p2_h", bufs=16) as hp, \
             tc.tile_pool(name="p2_ps", bufs=4, space="PSUM") as pp, \
             tc.tile_pool(name="p2_o", bufs=3) as op:
            wq = [wp.tile([128, D], F32R, tag=f"wq{_i}", name=f"wq{_i}") for _i in range(NDI)]
            for i in range(NDI):
                nc.sync.dma_start(wq[i][:], wqT[i * 128:(i + 1) * 128, :])
            for bc in (0, 2, 4, 6, 1, 3, 5, 7):
                ht = [hp.tile([128, SL], F32R, tag="ht", name=f"ht{_i}") for _i in range(NDI)]
                for i in range(NDI):
                    nc.sync.dma_start(ht[i][:], hsT[bc][i * 128:(i + 1) * 128, :])
                for j in range(NDO):
                    ps = pp.tile([128, SL], F32, tag="ps")
                    for i in range(NDI):
                        nc.tensor.matmul(ps[:], wq[i][:, j * 128:(j + 1) * 128],
                                         ht[i][:], start=(i == 0),
                                         stop=(i == NDI - 1))
                    qt = op.tile([128, SL], F32R, tag="qt")
                    nc.scalar.activation(
                        qt[:], ps[:], mybir.ActivationFunctionType.Identity,
                        bias=bq_t[:, j:j + 1], scale=SCALE)
                    nc.sync.dma_start(qT_s[bc][j], qt[:])

        # ================= P3: attention =================
        with tc.tile_pool(name="p3_in", bufs=18) as ip, \
             tc.tile_pool(name="p3_v", bufs=10) as vp, \
             tc.tile_pool(name="p3_e", bufs=2) as epool, \
             tc.tile_pool(name="p3_d", bufs=2) as dpool, \
             tc.tile_pool(name="p3_r", bufs=6) as rpool, \
             tc.tile_pool(name="p3_ps", bufs=2, space="PSUM") as pp, \
             tc.tile_pool(name="p3_ps2", bufs=1, space="PSUM") as pp2:
            for b in range(B):
                for hp_i in range(H // 2):          # head pair
                    # per-head loads keep every matmul operand at
                    # partition base 0 (non-zero PE tile positions are
                    # broken on this toolchain)
                    qt, kt, v0, v1 = [], [], [], []
                    for c in range(C):
                        bc = c * B + b
                        for j in range(2):
                            q = ip.tile([64, SL], F32R, tag="qt")
                            nc.sync.dma_start(
                                q[:], qT_s[bc][hp_i][j * 64:(j + 1) * 64, :])
                            qt.append(q)
                            k = ip.tile([64, T], F32R, tag="kt")
                            nc.sync.dma_start(
                                k[:], kT_s[hp_i][j * 64:(j + 1) * 64,
                                                 bc * TP:bc * TP + T])
                            kt.append(k)
                        va = vp.tile([128, 128], BF16, tag="v0")
                        nc.sync.dma_start(
                            va[:], v_s[bc][0:T0,
                                           hp_i * 128:(hp_i + 1) * 128])
                        v0.append(va)
                        vb = vp.tile([T1, 128], BF16, tag="v1")
                        nc.sync.dma_start(
                            vb[:], v_s[bc][T0:T,
                                           hp_i * 128:(hp_i + 1) * 128])
                        v1.append(vb)

                    # scores + exp for both heads of the pair
                    e0 = [epool.tile([128, 2 * SL], BF16, tag=f"e0_{_i}", name=f"e0_{_i}")
                          for _i in range(C)]
                    e1 = [epool.tile([T1, 2 * SL], BF16, tag=f"e1_{_i}", name=f"e1_{_i}")
                          for _i in range(C)]
                    for j in range(2):              # head within pair
                        for c in range(C):
                            qk, kk = qt[c * 2 + j], kt[c * 2 + j]
                            s0 = pp.tile([128, SL], F32, tag="s0")
                            nc.tensor.matmul(s0[:], kk[:, 0:T0], qk[:],
                                             start=True, stop=True)
                            nc.scalar.activation(
                                e0[c][:, j * SL:(j + 1) * SL], s0[:],
                                mybir.ActivationFunctionType.Exp)
                            s1 = pp.tile([T1, SL], F32, tag="s1")
                            nc.tensor.matmul(s1[:], kk[:, T0:T], qk[:],
                                             start=True, stop=True)
                            nc.scalar.activation(
                                e1[c][:, j * SL:(j + 1) * SL], s1[:],
                                mybir.ActivationFunctionType.Exp)

                    # component softmax: w_c = e_c / sum_c e_c  (in place;
                    # pure-bf16 adds/muls run on GpSimd to unload DVE)
                    for (ee, rows) in ((e0, 128), (e1, T1)):
                        t01 = dpool.tile([128, 2 * SL], BF16, tag="t01")
                        t23 = dpool.tile([128, 2 * SL], BF16, tag="t23")
                        dd = dpool.tile([128, 2 * SL], F32, tag="dd")
                        rbf = dpool.tile([128, 2 * SL], BF16, tag="rbf")
                        ENG_ADD.tensor_add(t01[0:rows, :], ee[0][:],
                                           ee[1][:])
                        ENG_ADD.tensor_add(t23[0:rows, :], ee[2][:],
                                           ee[3][:])
                        nc.vector.tensor_add(dd[0:rows, :], t01[0:rows, :],
                                             t23[0:rows, :])
                        nc.vector.reciprocal_approx_fast(dd[0:rows, :],
                                                         dd[0:rows, :])
                        nc.vector.tensor_copy(rbf[0:rows, :], dd[0:rows, :])
                        for c in range(C):
                            ENG_MUL.tensor_mul(ee[c][:], ee[c][:],
                                               rbf[0:rows, :])

                    # AV + rowsum + normalize per (c, head-pair packed wide)
                    for c in range(C):
                        bc = c * B + b
                        po = pp2.tile([64, 2 * SL], F32, tag="po")
                        pr = pp2.tile([64, 2 * SL], F32, tag="pr")
                        for j in range(2):
                            sl_ = slice(j * SL, (j + 1) * SL)
                            nc.tensor.matmul(
                                po[:, sl_], v0[c][:, j * 64:(j + 1) * 64],
                                e0[c][:, sl_], start=True, stop=False)
                            nc.tensor.matmul(
                                po[:, sl_], v1[c][:, j * 64:(j + 1) * 64],
                                e1[c][:, sl_], start=False, stop=True)
                            nc.tensor.matmul(
                                pr[:, sl_], ones_bf[0:128, 0:64],
                                e0[c][:, sl_], start=True, stop=False)
                            nc.tensor.matmul(
                                pr[:, sl_], ones_bf[0:T1, 0:64],
                                e1[c][:, sl_], start=False, stop=True)
                        rb = rpool.tile([64, 2 * SL], F32, tag="rb")
                        nc.vector.reciprocal_approx_fast(rb[:], pr[:])
                        at = rpool.tile([64, 2 * SL], F32R, tag="at")
                        nc.vector.tensor_mul(at[:], po[:], rb[:])
                        for j in range(2):
                            h = hp_i * 2 + j
                            nc.sync.dma_start(
                                at_s[bc][h * 64:(h + 1) * 64, :],
                                at[:, j * SL:(j + 1) * SL])

        # ================= P4: O projection =================
        with tc.tile_pool(name="p4_w", bufs=1) as wp, \
             tc.tile_pool(name="p4_a", bufs=16) as apool, \
             tc.tile_pool(name="p4_ps", bufs=4, space="PSUM") as pp, \
             tc.tile_pool(name="p4_o", bufs=3) as op:
            wo = [wp.tile([128, D], F32R, tag=f"wo{_i}", name=f"wo{_i}") for _i in range(NDI)]
            for i in range(NDI):
                nc.sync.dma_start(wo[i][:], woT[i * 128:(i + 1) * 128, :])
            for bc in (0, 2, 4, 6, 1, 3, 5, 7):
                att = [apool.tile([128, SL], F32R, tag="att", name=f"att{_i}")
                       for _i in range(NDI)]
                for i in range(NDI):
                    nc.sync.dma_start(att[i][:],
                                      at_s[bc][i * 128:(i + 1) * 128, :])
                for j in range(NDO):
                    ps = pp.tile([128, SL], F32, tag="ps")
                    for i in range(NDI):
                        nc.tensor.matmul(ps[:], wo[i][:, j * 128:(j + 1) * 128],
                                         att[i][:], start=(i == 0),
                                         stop=(i == NDI - 1))
                    ot = op.tile([128, SL], F32, tag="ot")
                    nc.scalar.activation(
                        ot[:], ps[:], mybir.ActivationFunctionType.Identity,
                        bias=bo_t[:, j:j + 1])
                    nc.sync.dma_start(outT[bc][j * 128:(j + 1) * 128, :],
                                      ot[:])

    nc.compile()
    return nc


_NC_CACHE = None


def _get_program():
    global _NC_CACHE
    if _NC_CACHE is None:
        _NC_CACHE = build_program()
    return _NC_CACHE


def make_in_maps(hidden_states, encoder_hidden_states, Wq, bq, Wk, bk,
                 Wv, bv, Wo, bo):
    """Host-side shard + transpose prep. Returns per-core input dicts."""
    hs = np.ascontiguousarray(hidden_states, dtype=np.float32)
    ehs = np.ascontiguousarray(encoder_hidden_states, dtype=np.float32)

    ehsT = np.zeros((D, KV_N), dtype=np.float32)
    for bc in range(BC):
        ehsT[:, bc * TP:bc * TP + T] = ehs[bc].T

    shared = {
        "ehsT": ehsT,
        "wqT": np.ascontiguousarray(Wq.T, dtype=np.float32),
        "wkT": np.ascontiguousarray(Wk.T, dtype=np.float32),
        "wvT": np.ascontiguousarray(Wv.T, dtype=np.float32),
        "woT": np.ascontiguousarray(Wo.T, dtype=np.float32),
        "bqs": np.ascontiguousarray(
            (np.asarray(bq, np.float32) * SCALE).reshape(NDO, 128).T),
        "bks": np.ascontiguousarray(
            np.asarray(bk, np.float32).reshape(NDO, 128).T),
        "bos": np.ascontiguousarray(
            np.asarray(bo, np.float32).reshape(NDO, 128).T),
        "bvr": np.asarray(bv, np.float32).reshape(1, D),
        "onesr": np.ones((1, 128), np.float32),
    }
    in_maps = []
    for core in range(NCORES):
        sl = slice(core * SL, (core + 1) * SL)
        hsT = np.ascontiguousarray(hs[:, sl, :].transpose(0, 2, 1))
        in_maps.append({**shared, "hsT": hsT})
    return in_maps


def run_sharded(inputs, trace=False, tmpdir=None, trace_cores=None):
    from concourse.bass_utils import run_bass_kernel_spmd
    nc = _get_program()
    in_maps = make_in_maps(**inputs)
    res = run_bass_kernel_spmd(nc, in_maps, list(range(NCORES)), trace=trace,
                               tmpdir=tmpdir, trace_cores=trace_cores)
    out = np.empty((BC, S, D), dtype=np.float32)
    for core in range(NCORES):
        sl = slice(core * SL, (core + 1) * SL)
        out[:, sl, :] = res.results[core]["outT"].transpose(0, 2, 1)
    return out, res


def kernel(**inputs):
    out, _ = run_sharded(inputs, trace=False)
    return out



# revision 12
# speedup vs baseline: 1.2103x; 1.2103x over previous
"""Trainium2 Bass kernel for nn_DecomposingAttnProcessor.

Math (reference):
    q = hs @ Wq.T + bq;  k = ehs @ Wk.T + bk;  v = ehs @ Wv.T + bv
    scores = (q @ k.T) * dh**-0.5 per (bc, head)      [BC, H, S, T]
    w = softmax(scores over the COMPONENT axis)        (bc = c*B + b, C=4, B=2)
    w = w / (sum_t w + eps)
    out = (w @ v) -> [BC, S, D] -> @ Wo.T + bo

Distribution: shard S (4096 query tokens) across 8 cores, 512 each. Every
core handles all (bc, head) pairs for its S-slice, so the component softmax
group (same b, all c) stays on one core and each core emits complete output
rows (no cross-core reduction).

v2 design:
  - all matmul operands bf16 (FWL-capable weight loads, half DMA/SBUF),
    fp32 PSUM accumulation, fp32 biases.
  - rowsum folded into the AV matmul: V tiles carry a 65th ones-column per
    head so po[64] = sum_t w; normalize = DVE recip + GpSimd
    partition_broadcast + DVE mul.  No separate ones-matmuls.
  - V projection packed densely over all bc (1232 rows, chunks cross bc
    boundaries) instead of per-(bc, T-chunk) padded matmuls.
  - program emission interleaves the Q projection of batch b=1 into the
    attention loop of b=0, and the O projection of b=0 into attention of
    b=1, so the PE stays warm and busy through the attention phase.
"""

import numpy as np
from contextlib import ExitStack

import concourse.bass as bass
import concourse.tile as tile
from concourse import bacc, mybir

F32 = mybir.dt.float32
BF16 = mybir.dt.bfloat16

# problem shape (hardcoded per contract)
BC, S, D = 8, 4096, 1536
T = 154
C, B = 4, 2
H, DH = 24, 64
NCORES = 8
SL = S // NCORES          # 512 S-rows per core
NDI = D // 128            # 12 din chunks
NDO = D // 128            # 12 dout tiles
KVN = BC * T              # 1232 packed kv rows (no padding)
T0, T1 = 128, T - 128     # T chunks: 128 + 26
SCALE = DH ** -0.5
HPP = H // 2              # 12 head pairs
W2 = 2 * SL               # 1024: free width of a (c) block (both heads)

KV_NBLK = [(0, 512), (512, 512), (1024, 208)]    # N-blocks for kT proj
DO_NBLK = [(0, 512), (512, 512), (1024, 512)]    # N-blocks over D for v proj
VCHUNKS = [(i * 128, min(128, KVN - i * 128)) for i in range((KVN + 127) // 128)]


def build_program():
    nc = bacc.Bacc("TRN2", target_bir_lowering=False, debug=False)

    # ---- external I/O (per core) ----
    hsT = nc.dram_tensor("hsT", [BC, D, SL], BF16, kind="ExternalInput").ap()
    ehsT = nc.dram_tensor("ehsT", [D, KVN], BF16, kind="ExternalInput").ap()
    wqT = nc.dram_tensor("wqT", [D, D], BF16, kind="ExternalInput").ap()
    wkT = nc.dram_tensor("wkT", [D, D], BF16, kind="ExternalInput").ap()
    wvT = nc.dram_tensor("wvT", [D, D], BF16, kind="ExternalInput").ap()
    woT = nc.dram_tensor("woT", [D, D], BF16, kind="ExternalInput").ap()
    # biases laid out [128, 12] on host (bq pre-scaled by dh**-0.5)
    bqs = nc.dram_tensor("bqs", [128, NDO], F32, kind="ExternalInput").ap()
    bks = nc.dram_tensor("bks", [128, NDO], F32, kind="ExternalInput").ap()
    bos = nc.dram_tensor("bos", [128, NDO], F32, kind="ExternalInput").ap()
    bvr = nc.dram_tensor("bvr", [1, D], BF16, kind="ExternalInput").ap()
    onesr = nc.dram_tensor("onesr", [1, 128], BF16, kind="ExternalInput").ap()
    outT = nc.dram_tensor("outT", [BC, D, SL], F32, kind="ExternalOutput").ap()

    # ---- DRAM scratch ----
    import os
    _dbg = os.environ.get("DK_DEBUG", "0") == "1"
    _kind = dict(kind="ExternalOutput") if _dbg else {}
    kT_s = [nc.dram_tensor(f"kT_s{j}", [128, KVN], BF16, **_kind).ap()
            for j in range(HPP)]
    v_s = nc.dram_tensor("v_s", [KVN, D], BF16, **_kind).ap()
    qT_s = [nc.dram_tensor(f"qT_s{bc}", [NDO, 128, SL], BF16, **_kind).ap()
            for bc in range(BC)]
    at_s = [nc.dram_tensor(f"at_s{bc}", [D, SL], BF16, **_kind).ap()
            for bc in range(BC)]
    if _dbg:
        po_dbg = nc.dram_tensor("po_dbg", [1, 2 * SL], F32,
                                kind="ExternalOutput").ap()
        e0_dbg = nc.dram_tensor("e0_dbg", [T0, C * 2 * SL], BF16,
                                kind="ExternalOutput").ap()
        dd_dbg = nc.dram_tensor("dd_dbg", [T0, 2 * SL], F32,
                                kind="ExternalOutput").ap()
        rr_dbg = nc.dram_tensor("rr_dbg", [1, 2 * SL], F32,
                                kind="ExternalOutput").ap()
        rb_dbg = nc.dram_tensor("rb_dbg", [64, 2 * SL], F32,
                                kind="ExternalOutput").ap()
        vt_dbg = nc.dram_tensor("vt_dbg", [T0, 130], BF16,
                                kind="ExternalOutput").ap()

    with tile.TileContext(nc) as tc, ExitStack() as ctx:
        # ---------- persistent pools ----------
        const = ctx.enter_context(tc.tile_pool(name="const", bufs=1))
        # PSUM pools: s0 1bk + s1 1bk + po 2x2bk + proj 2x1bk = 8 banks
        ps_s0 = ctx.enter_context(tc.tile_pool(name="ps_s0", bufs=1,
                                               space="PSUM"))
        ps_s1 = ctx.enter_context(tc.tile_pool(name="ps_s1", bufs=1,
                                               space="PSUM"))
        ps_po = ctx.enter_context(tc.tile_pool(name="ps_po", bufs=2,
                                               space="PSUM"))
        ps_pj = ctx.enter_context(tc.tile_pool(name="ps_pj", bufs=2,
                                               space="PSUM"))

        # ---------- constants ----------
        bq_t = const.tile([128, NDO], F32)
        bk_t = const.tile([128, NDO], F32)
        bo_t = const.tile([128, NDO], F32)
        bv_t = const.tile([1, D], BF16)
        ones_row = const.tile([1, 128], BF16)
        nc.sync.dma_start(bq_t[:], bqs[:])
        nc.sync.dma_start(bk_t[:], bks[:])
        nc.sync.dma_start(bo_t[:], bos[:])
        nc.sync.dma_start(bv_t[:], bvr[:])
        nc.sync.dma_start(ones_row[:], onesr[:])

        # ================= P1: K and V projections =================
        with tc.tile_pool(name="p1w", bufs=1) as p1w, \
             tc.tile_pool(name="p1o", bufs=2) as p1o:
            eh = p1w.tile([128, NDI * KVN], BF16, name="eh")
            wk = p1w.tile([128, NDI * D], BF16, name="wk")
            wv = p1w.tile([128, NDI * D], BF16, name="wv")
            nc.sync.dma_start(
                eh[:], ehsT[:].rearrange("(i p) n -> p i n", p=128))
            nc.sync.dma_start(
                wk[:], wkT[:].rearrange("(i p) n -> p i n", p=128))
            nc.sync.dma_start(
                wv[:], wvT[:].rearrange("(i p) n -> p i n", p=128))

            # ---- P1a: kT = Wk @ ehs^T (+bk), all bc packed ----
            for j in range(HPP):          # 12 head-pair row blocks
                ot = p1o.tile([128, KVN], BF16, tag="kot")
                for (nb0, nbl) in KV_NBLK:
                    ps = ps_pj.tile([128, 512], F32, tag="ps")
                    for i in range(NDI):
                        nc.tensor.matmul(
                            ps[:, 0:nbl],
                            wk[:, i * D + j * 128:i * D + (j + 1) * 128],
                            eh[:, i * KVN + nb0:i * KVN + nb0 + nbl],
                            start=(i == 0), stop=(i == NDI - 1))
                    nc.scalar.activation(
                        ot[:, nb0:nb0 + nbl], ps[:, 0:nbl],
                        mybir.ActivationFunctionType.Identity,
                        bias=bk_t[:, j:j + 1])
                nc.sync.dma_start(kT_s[j][:], ot[:])

            # ---- P1b: v = ehs @ Wv.T (+bv), rows packed across bc ----
            for (ck0, ckl) in VCHUNKS:
                ot = p1o.tile([128, D], BF16, tag="vot")
                for (nb0, nbl) in DO_NBLK:
                    ps = ps_pj.tile([128, 512], F32, tag="ps")
                    for i in range(NDI):
                        nc.tensor.matmul(
                            ps[0:ckl, :],
                            eh[:, i * KVN + ck0:i * KVN + ck0 + ckl],
                            wv[:, i * D + nb0:i * D + nb0 + nbl],
                            start=(i == 0), stop=False)
                    nc.tensor.matmul(
                        ps[0:ckl, :],
                        ones_row[0:1, 0:ckl],
                        bv_t[0:1, nb0:nb0 + nbl],
                        start=False, stop=True,
                        skip_group_check=True)
                    nc.scalar.copy(ot[0:ckl, nb0:nb0 + nbl], ps[0:ckl, :])
                nc.sync.dma_start(v_s[ck0:ck0 + ckl, :], ot[0:ckl, :])

        # ---------- P3 pools (reuse P1's SBUF space) ----------
        p3qk = ctx.enter_context(tc.tile_pool(name="p3qk", bufs=2))
        p3v = ctx.enter_context(tc.tile_pool(name="p3v", bufs=1))
        p3e = ctx.enter_context(tc.tile_pool(name="p3e", bufs=2))
        p3d = ctx.enter_context(tc.tile_pool(name="p3d", bufs=2))
        p3r = ctx.enter_context(tc.tile_pool(name="p3r", bufs=2))
        p3at = ctx.enter_context(tc.tile_pool(name="p3at", bufs=2))

        # persistent V tiles: [T', 130] = [head0 v | ones | head1 v | ones];
        # the ones columns (64, 129) are written once and never re-DMA'd.
        vt0 = [[p3v.tile([T0, 130], BF16, tag=f"vt0_{c}_{k}",
                         name=f"vt0_{c}_{k}") for k in range(2)]
               for c in range(C)]
        vt1 = [[p3v.tile([T1, 130], BF16, tag=f"vt1_{c}_{k}",
                         name=f"vt1_{c}_{k}") for k in range(2)]
               for c in range(C)]
        for c in range(C):
            for k in range(2):
                for vt in (vt0[c][k], vt1[c][k]):
                    nc.vector.memset(vt[:, 64:65], 1.0)
                    nc.vector.memset(vt[:, 129:130], 1.0)

        # ================= P2/P4 emitters =================
        def emit_qproj(wq, hpool, opool, bc):
            ht = hpool.tile([128, NDI * SL], BF16, tag="ht")
            nc.sync.dma_start(
                ht[:], hsT[bc].rearrange("(i p) n -> p i n", p=128))
            for j in range(NDO):
                ps = ps_pj.tile([128, SL], F32, tag="ps")
                for i in range(NDI):
                    nc.tensor.matmul(
                        ps[:], wq[:, i * D + j * 128:i * D + (j + 1) * 128],
                        ht[:, i * SL:(i + 1) * SL],
                        start=(i == 0), stop=(i == NDI - 1))
                qt = opool.tile([128, SL], BF16, tag="qt")
                nc.scalar.activation(
                    qt[:], ps[:], mybir.ActivationFunctionType.Identity,
                    bias=bq_t[:, j:j + 1], scale=SCALE)
                nc.sync.dma_start(qT_s[bc][j], qt[:])

        def emit_oproj(wo, apool, opool, bc):
            att = apool.tile([128, NDI * SL], BF16, tag="att")
            nc.sync.dma_start(
                att[:], at_s[bc][:].rearrange("(i p) n -> p i n", p=128))
            for j in range(NDO):
                ps = ps_pj.tile([128, SL], F32, tag="ps")
                for i in range(NDI):
                    nc.tensor.matmul(
                        ps[:], wo[:, i * D + j * 128:i * D + (j + 1) * 128],
                        att[:, i * SL:(i + 1) * SL],
                        start=(i == 0), stop=(i == NDI - 1))
                ot = opool.tile([128, SL], F32, tag="ot")
                nc.scalar.activation(
                    ot[:], ps[:], mybir.ActivationFunctionType.Identity,
                    bias=bo_t[:, j:j + 1])
                nc.sync.dma_start(outT[bc][j * 128:(j + 1) * 128, :], ot[:])

        # ================= P3 emitter =================
        def emit_attn(b, hp):
            par = hp % 2
            qt, kt = [], []
            for c in range(C):
                bc = c * B + b
                for j in range(2):
                    q = p3qk.tile([64, SL], BF16, tag=f"q{c}{j}")
                    nc.sync.dma_start(
                        q[:], qT_s[bc][hp][j * 64:(j + 1) * 64, :])
                    qt.append(q)
                    k = p3qk.tile([64, T], BF16, tag=f"k{c}{j}")
                    nc.sync.dma_start(
                        k[:], kT_s[hp][j * 64:(j + 1) * 64,
                                       bc * T:(bc + 1) * T])
                    kt.append(k)
                # v: both heads' 64 cols -> tile cols {0:64, 65:129}
                nc.sync.dma_start(
                    vt0[c][par][:, 0:130]
                    .rearrange("t (j d) -> t j d", j=2)[:, :, 0:64],
                    v_s[bc * T:bc * T + T0, hp * 128:(hp + 1) * 128]
                    .rearrange("t (j d) -> t j d", j=2))
                nc.sync.dma_start(
                    vt1[c][par][:, 0:130]
                    .rearrange("t (j d) -> t j d", j=2)[:, :, 0:64],
                    v_s[bc * T + T0:bc * T + T, hp * 128:(hp + 1) * 128]
                    .rearrange("t (j d) -> t j d", j=2))

            # scores + exp; e layout [T', C*W2] = (c, j, s) c-major
            e0 = p3e.tile([T0, C * W2], BF16, tag="e0")
            e1 = p3e.tile([T1, C * W2], BF16, tag="e1")
            for c in range(C):
                for j in range(2):
                    qk, kk = qt[c * 2 + j], kt[c * 2 + j]
                    sl_e = slice(c * W2 + j * SL, c * W2 + (j + 1) * SL)
                    s0 = ps_s0.tile([T0, SL], F32, tag="s0")
                    nc.tensor.matmul(s0[:], kk[:, 0:T0], qk[:],
                                     start=True, stop=True)
                    nc.scalar.activation(
                        e0[:, sl_e], s0[:],
                        mybir.ActivationFunctionType.Exp)
                    s1 = ps_s1.tile([T1, SL], F32, tag="s1")
                    nc.tensor.matmul(s1[:], kk[:, T0:T], qk[:],
                                     start=True, stop=True)
                    nc.scalar.activation(
                        e1[:, sl_e], s1[:],
                        mybir.ActivationFunctionType.Exp)

            # component softmax: d = sum_c e_c, e_c *= 1/d
            for (ee, rows) in ((e0, T0), (e1, T1)):
                dd = p3d.tile([rows, W2], F32, tag=f"d{rows}")
                nc.vector.tensor_add(dd[:], ee[:, 0:W2], ee[:, W2:2 * W2])
                nc.vector.tensor_add(dd[:], dd[:], ee[:, 2 * W2:3 * W2])
                nc.vector.tensor_add(dd[:], dd[:], ee[:, 3 * W2:4 * W2])
                nc.vector.reciprocal_approx_fast(dd[:], dd[:])
                if _dbg and b == 0 and hp == 0 and rows == T0:
                    nc.sync.dma_start(dd_dbg[:], dd[:])
                db = p3d.tile([rows, W2], BF16, tag=f"db{rows}")
                nc.vector.tensor_copy(db[:], dd[:])
                for c in range(C):
                    sl_ = slice(c * W2, (c + 1) * W2)
                    eng = nc.gpsimd if rows == T0 else nc.vector
                    eng.tensor_mul(ee[:, sl_], ee[:, sl_], db[:])

            if _dbg and b == 0 and hp == 0:
                nc.sync.dma_start(e0_dbg[:], e0[:])
                nc.sync.dma_start(vt_dbg[:], vt0[0][par][:])

            # AV with folded rowsum + normalize, per component
            for c in range(C):
                bc = c * B + b
                po = ps_po.tile([65, W2], F32, tag="po")
                for j in range(2):
                    sl_ = slice(j * SL, (j + 1) * SL)
                    esl = slice(c * W2 + j * SL, c * W2 + (j + 1) * SL)
                    nc.tensor.matmul(po[:, sl_],
                                     vt0[c][par][:, j * 65:(j + 1) * 65],
                                     e0[:, esl], start=True, stop=False)
                    nc.tensor.matmul(po[:, sl_],
                                     vt1[c][par][:, j * 65:(j + 1) * 65],
                                     e1[:, esl], start=False, stop=True)
                rr0 = p3r.tile([1, W2], F32, tag="rr0")
                nc.vector.tensor_copy(rr0[:], po[64:65, :])
                nc.vector.reciprocal_approx_fast(rr0[:], rr0[:])
                rb = p3r.tile([64, W2], F32, tag="rb")
                nc.gpsimd.partition_broadcast(rb[:], rr0[:])
                at = p3at.tile([64, W2], BF16, tag="at")
                nc.vector.tensor_mul(at[:], po[0:64, :], rb[:])
                if _dbg and b == 0 and hp == 0 and c == 0:
                    nc.sync.dma_start(rr_dbg[:], rr0[:])
                    pox = p3r.tile([65, W2], F32, tag="pox")
                    nc.scalar.copy(pox[64:65, :], po[64:65, :])
                    nc.sync.dma_start(po_dbg[:], pox[64:65, :])
                    nc.sync.dma_start(rb_dbg[:], rb[:])
                for j in range(2):
                    h = hp * 2 + j
                    nc.sync.dma_start(
                        at_s[bc][h * 64:(h + 1) * 64, :],
                        at[:, j * SL:(j + 1) * SL])

        # ================= schedule =================
        with tc.tile_pool(name="p2w", bufs=1) as p2w, \
             tc.tile_pool(name="p2h", bufs=2) as p2h, \
             tc.tile_pool(name="p2o", bufs=3) as p2o:
            wq = p2w.tile([128, NDI * D], BF16, name="wq")
            nc.sync.dma_start(
                wq[:], wqT[:].rearrange("(i p) n -> p i n", p=128))
            for bc in (0, 2, 4, 6):
                emit_qproj(wq, p2h, p2o, bc)
            # window 2: attention b=0 interleaved with Q proj b=1
            for hp in range(HPP):
                if hp % 3 == 0:
                    emit_qproj(wq, p2h, p2o, (1, 3, 5, 7)[hp // 3])
                emit_attn(0, hp)

        with tc.tile_pool(name="p4w", bufs=1) as p4w, \
             tc.tile_pool(name="p4a", bufs=2) as p4a, \
             tc.tile_pool(name="p4o", bufs=3) as p4o:
            wo = p4w.tile([128, NDI * D], BF16, name="wo")
            nc.sync.dma_start(
                wo[:], woT[:].rearrange("(i p) n -> p i n", p=128))
            # window 3: attention b=1 interleaved with O proj b=0
            for hp in range(HPP):
                emit_attn(1, hp)
                if hp % 3 == 2:
                    emit_oproj(wo, p4a, p4o, (0, 2, 4, 6)[hp // 3])
            for bc in (1, 3, 5, 7):
                emit_oproj(wo, p4a, p4o, bc)

    nc.compile()
    return nc


_NC_CACHE = None


def _get_program():
    global _NC_CACHE
    if _NC_CACHE is None:
        _NC_CACHE = build_program()
    return _NC_CACHE


def make_in_maps(hidden_states, encoder_hidden_states, Wq, bq, Wk, bk,
                 Wv, bv, Wo, bo):
    """Host-side shard + transpose prep. Returns per-core input dicts."""
    import ml_dtypes
    bf16 = ml_dtypes.bfloat16
    hs = np.ascontiguousarray(hidden_states, dtype=np.float32)
    ehs = np.ascontiguousarray(encoder_hidden_states, dtype=np.float32)

    # ehsT [D, KVN]: all bc packed contiguously, no padding
    ehsT = np.ascontiguousarray(
        ehs.transpose(2, 0, 1).reshape(D, KVN)).astype(bf16)

    shared = {
        "ehsT": ehsT,
        "wqT": np.ascontiguousarray(Wq.T).astype(bf16),
        "wkT": np.ascontiguousarray(Wk.T).astype(bf16),
        "wvT": np.ascontiguousarray(Wv.T).astype(bf16),
        "woT": np.ascontiguousarray(Wo.T).astype(bf16),
        "bqs": np.ascontiguousarray(
            (np.asarray(bq, np.float32) * SCALE).reshape(NDO, 128).T),
        "bks": np.ascontiguousarray(
            np.asarray(bk, np.float32).reshape(NDO, 128).T),
        "bos": np.ascontiguousarray(
            np.asarray(bo, np.float32).reshape(NDO, 128).T),
        "bvr": np.asarray(bv, np.float32).reshape(1, D).astype(bf16),
        "onesr": np.ones((1, 128), np.float32).astype(bf16),
    }
    in_maps = []
    for core in range(NCORES):
        sl = slice(core * SL, (core + 1) * SL)
        hsT = np.ascontiguousarray(hs[:, sl, :].transpose(0, 2, 1)
                                   ).astype(bf16)
        in_maps.append({**shared, "hsT": hsT})
    return in_maps


def run_sharded(inputs, trace=False, tmpdir=None, trace_cores=None):
    from concourse.bass_utils import run_bass_kernel_spmd
    nc = _get_program()
    in_maps = make_in_maps(**inputs)
    res = run_bass_kernel_spmd(nc, in_maps, list(range(NCORES)), trace=trace,
                               tmpdir=tmpdir, trace_cores=trace_cores)
    out = np.empty((BC, S, D), dtype=np.float32)
    for core in range(NCORES):
        sl = slice(core * SL, (core + 1) * SL)
        out[:, sl, :] = res.results[core]["outT"].transpose(0, 2, 1)
    return out, res


def kernel(**inputs):
    out, _ = run_sharded(inputs, trace=False)
    return out
